# revision 8
# baseline (speedup 1.0000x reference)
"""Trainium2 Bass kernel for CustomKernelConvLatticeIm2Row (gnn message passing).

Full inputs in, full outputs out. Sharding: vertex dim N split into 8 equal
shards (padded to 300032 = 8 * 37504); hidden_state replicated so each core
gathers its neighbors locally (no cross-core communication).

Per 128xT vertex tile (partition p owns T consecutive vertices):
  - load neighbor_idx + lattice slices, clip indices to >=0, valid = idx >= 0
  - T*9 indirect DMAs, each gathering 128 neighbor rows (256B each, one row
    per partition - the HW dynamic-DMA ucode consumes one index per
    partition per instruction)
  - diff -> square -> reduce(F) -> sqrt -> mask -> normalize -> AFLOW weights
  - weighted neighbor sum via mult + strided reduce(K), + bias
"""

import numpy as np

N = 300000
F = 64
K = 9
P = 128
NCORES = 8
VPAD = 300032  # next multiple of 8*128 covering N
VSHARD = VPAD // NCORES  # 37504 = 128 * 293
TMAX = 8

_cache = {}


def _tiles_for(V, tmax):
    assert V % P == 0
    out = []
    base = 0
    blocks = V // P
    while blocks > 0:
        t = min(tmax, blocks)
        out.append((base, t))
        base += P * t
        blocks -= t
    return out


def _build_nc(vshard=VSHARD, nhid=N, tmax=TMAX):
    from contextlib import ExitStack
    import concourse.bass as bass
    import concourse.bacc as bacc
    import concourse.mybir as mybir
    import concourse.tile as tile

    f32 = mybir.dt.float32
    i32 = mybir.dt.int32

    nc = bacc.Bacc("TRN2", debug=False)
    table_d = nc.dram_tensor("table", [nhid, F], f32, kind="ExternalInput").ap()
    lat_d = nc.dram_tensor("lat", [vshard, F], f32, kind="ExternalInput").ap()
    nidx_d = nc.dram_tensor("nidx", [vshard, K], i32, kind="ExternalInput").ap()
    bias_d = nc.dram_tensor("bias", [1, F], f32, kind="ExternalInput").ap()
    alpha_d = nc.dram_tensor("alpha", [1, 1], f32, kind="ExternalInput").ap()
    beta_d = nc.dram_tensor("beta", [1, 1], f32, kind="ExternalInput").ap()
    aflow_d = nc.dram_tensor("aflow", [vshard, F], f32, kind="ExternalOutput").ap()
    w_d = nc.dram_tensor("w", [vshard, K], f32, kind="ExternalOutput").ap()

    with tile.TileContext(nc) as tc, ExitStack() as ctx:
        const_pool = ctx.enter_context(tc.tile_pool(name="const", bufs=1))
        io_pool = ctx.enter_context(tc.tile_pool(name="io", bufs=4))
        g_pool = ctx.enter_context(tc.tile_pool(name="gbuf", bufs=4))
        d_pool = ctx.enter_context(tc.tile_pool(name="dbuf", bufs=2))
        sm_pool = ctx.enter_context(tc.tile_pool(name="small", bufs=4))

        bias_sb = const_pool.tile([P, F], f32)
        nc.sync.dma_start(out=bias_sb[:, :], in_=bias_d[:, :].partition_broadcast(P).squeeze(1))
        alpha_sb = const_pool.tile([P, 1], f32)
        nc.sync.dma_start(out=alpha_sb[:, :], in_=alpha_d[:, :].partition_broadcast(P).squeeze(1))
        beta_sb = const_pool.tile([P, 1], f32)
        nc.sync.dma_start(out=beta_sb[:, :], in_=beta_d[:, :].partition_broadcast(P).squeeze(1))
        negbeta_sb = const_pool.tile([P, 1], f32)
        nc.vector.tensor_scalar_mul(negbeta_sb[:, :], beta_sb[:, :], -1.0)
        ab_sb = const_pool.tile([P, 1], f32)
        nc.vector.tensor_mul(ab_sb[:, :], alpha_sb[:, :], beta_sb[:, :])

        for base, T in _tiles_for(vshard, tmax):
            TK = T * K
            TKF = T * K * F
            rows = slice(base, base + P * T)

            idx_t = io_pool.tile([P, TK], i32, tag="idx")
            nc.sync.dma_start(out=idx_t[:, :], in_=nidx_d[rows, :].rearrange("(p t) k -> p (t k)", p=P))
            lat_t = io_pool.tile([P, T * F], f32, tag="lat")
            nc.sync.dma_start(out=lat_t[:, :], in_=lat_d[rows, :].rearrange("(p t) f -> p (t f)", p=P))

            iz_t = sm_pool.tile([P, TK], i32, tag="iz")
            nc.gpsimd.tensor_scalar_max(iz_t[:, :], idx_t[:, :], 0)

            valid_t = sm_pool.tile([P, TK], f32, tag="valid")
            nc.vector.tensor_scalar(valid_t[:, :], idx_t[:, :], -0.5, None, mybir.AluOpType.is_gt)

            g_t = g_pool.tile([P, TKF], f32, tag="G")
            for c in range(TK):
                nc.gpsimd.indirect_dma_start(
                    out=g_t[:, c * F:(c + 1) * F],
                    out_offset=None,
                    in_=table_d[:, :],
                    in_offset=bass.IndirectOffsetOnAxis(ap=iz_t[:, c:c + 1], axis=0),
                )

            g4 = g_t[:, :].rearrange("p (t k f) -> p t k f", k=K, f=F)
            l4 = lat_t[:, :].rearrange("p (t f) -> p t f", f=F).unsqueeze(2).to_broadcast([P, T, K, F])

            d_t = d_pool.tile([P, TKF], f32, tag="D")
            nc.vector.tensor_tensor(
                out=d_t[:, :].rearrange("p (t k f) -> p t k f", k=K, f=F),
                in0=g4, in1=l4, op=mybir.AluOpType.subtract)
            nc.scalar.square(d_t[:, :], d_t[:, :])
            dist_t = sm_pool.tile([P, TK], f32, tag="dist")
            nc.vector.tensor_reduce(
                out=dist_t[:, :].unsqueeze(2),
                in_=d_t[:, :].rearrange("p (tk f) -> p tk f", f=F),
                axis=mybir.AxisListType.X, op=mybir.AluOpType.add)
            nc.scalar.sqrt(dist_t[:, :], dist_t[:, :])
            nc.vector.tensor_mul(dist_t[:, :], dist_t[:, :], valid_t[:, :])
            rsum_t = sm_pool.tile([P, T], f32, tag="rsum")
            nc.vector.tensor_reduce(
                out=rsum_t[:, :].unsqueeze(2),
                in_=dist_t[:, :].rearrange("p (t k) -> p t k", k=K),
                axis=mybir.AxisListType.X, op=mybir.AluOpType.add)
            recip_t = sm_pool.tile([P, T], f32, tag="recip")
            nc.vector.reciprocal(recip_t[:, :], rsum_t[:, :])
            w_t = sm_pool.tile([P, TK], f32, tag="w")
            nc.vector.tensor_tensor(
                out=w_t[:, :].rearrange("p (t k) -> p t k", k=K),
                in0=dist_t[:, :].rearrange("p (t k) -> p t k", k=K),
                in1=recip_t[:, :].unsqueeze(2).to_broadcast([P, T, K]),
                op=mybir.AluOpType.mult)
            nc.vector.tensor_scalar(w_t[:, :], w_t[:, :], alpha_sb[:, :1], None, mybir.AluOpType.min)
            nc.scalar.activation(w_t[:, :], w_t[:, :], mybir.ActivationFunctionType.Identity,
                                 bias=ab_sb[:, :1], scale=negbeta_sb[:, :1])
            nc.vector.tensor_mul(w_t[:, :], w_t[:, :], valid_t[:, :])
            nc.sync.dma_start(out=w_d[rows, :].rearrange("(p t) k -> p (t k)", p=P), in_=w_t[:, :])

            p_t = d_pool.tile([P, TKF], f32, tag="D")
            nc.vector.tensor_tensor(
                out=p_t[:, :].rearrange("p (t k f) -> p t k f", k=K, f=F),
                in0=g4,
                in1=w_t[:, :].rearrange("p (t k) -> p t k", k=K).unsqueeze(3).to_broadcast([P, T, K, F]),
                op=mybir.AluOpType.mult)
            af_t = io_pool.tile([P, T * F], f32, tag="af")
            nc.vector.tensor_reduce(
                out=af_t[:, :].rearrange("p (t f) -> p t f", f=F),
                in_=p_t[:, :].rearrange("p (t k f) -> p t f k", k=K, f=F),
                axis=mybir.AxisListType.X, op=mybir.AluOpType.add)
            nc.vector.tensor_tensor(
                out=af_t[:, :].rearrange("p (t f) -> p t f", f=F),
                in0=af_t[:, :].rearrange("p (t f) -> p t f", f=F),
                in1=bias_sb[:, :].unsqueeze(1).to_broadcast([P, T, F]),
                op=mybir.AluOpType.add)
            nc.scalar.dma_start(out=aflow_d[rows, :].rearrange("(p t) f -> p (t f)", p=P), in_=af_t[:, :])

    nc.compile()
    return nc


def _get_nc():
    if "nc" not in _cache:
        _cache["nc"] = _build_nc()
    return _cache["nc"]


def _run(in_maps, trace=False):
    from concourse.bass_utils import run_bass_kernel_spmd

    nc = _get_nc()
    return run_bass_kernel_spmd(nc, in_maps, core_ids=list(range(NCORES)), trace=trace)


def make_in_maps(lattice_values, hidden_state, bias, alpha, beta, neighbor_idx):
    lat = np.ascontiguousarray(np.asarray(lattice_values, dtype=np.float32))
    hid = np.ascontiguousarray(np.asarray(hidden_state, dtype=np.float32))
    nidx = np.ascontiguousarray(np.asarray(neighbor_idx, dtype=np.int32))
    bias = np.ascontiguousarray(np.asarray(bias, dtype=np.float32)).reshape(1, F)
    alpha = np.asarray(alpha, dtype=np.float32).reshape(1, 1)
    beta = np.asarray(beta, dtype=np.float32).reshape(1, 1)

    pad = VPAD - N
    lat_p = np.concatenate([lat, np.zeros((pad, F), np.float32)], axis=0)
    nidx_p = np.concatenate([nidx, np.zeros((pad, K), np.int32)], axis=0)

    in_maps = []
    for i in range(NCORES):
        r = slice(i * VSHARD, (i + 1) * VSHARD)
        in_maps.append({
            "table": hid,
            "lat": np.ascontiguousarray(lat_p[r]),
            "nidx": np.ascontiguousarray(nidx_p[r]),
            "bias": bias,
            "alpha": alpha,
            "beta": beta,
        })
    return in_maps


def kernel(lattice_values, hidden_state, bias, alpha, beta, neighbor_idx):
    nidx = np.ascontiguousarray(np.asarray(neighbor_idx, dtype=np.int32))
    in_maps = make_in_maps(lattice_values, hidden_state, bias, alpha, beta, nidx)
    res = _run(in_maps).results
    aflow = np.concatenate([r["aflow"] for r in res], axis=0)[:N]
    w = np.concatenate([r["w"] for r in res], axis=0)[:N]
    return aflow, w, nidx


# revision 9
# speedup vs baseline: 79.9969x; 79.9969x over previous
"""Trainium2 Bass kernel for CustomKernelConvLatticeIm2Row (gnn message passing).

Full inputs in, full outputs out. Sharding: vertex dim N split into 8 equal
shards (padded to 300032 = 8 * 37504); hidden_state replicated so each core
gathers its neighbors locally (no cross-core communication).

Per 128xT vertex tile (partition p owns T consecutive vertices):
  - load neighbor_idx + lattice slices, clip indices to >=0, valid = idx >= 0
  - T*9 indirect DMAs, each gathering 128 neighbor rows (256B each, one row
    per partition - the HW dynamic-DMA ucode consumes one index per
    partition per instruction)
  - diff -> square -> reduce(F) -> sqrt -> mask -> normalize -> AFLOW weights
  - weighted neighbor sum via mult + strided reduce(K), + bias
"""

import numpy as np

N = 300000
F = 64
K = 9
P = 128
NCORES = 8
VPAD = 300032  # next multiple of 8*128 covering N
VSHARD = VPAD // NCORES  # 37504 = 128 * 293
TMAX = 8

_cache = {}


def _tiles_for(V, tmax):
    assert V % P == 0
    out = []
    base = 0
    blocks = V // P
    while blocks > 0:
        t = min(tmax, blocks)
        out.append((base, t))
        base += P * t
        blocks -= t
    return out


def _build_nc(vshard=VSHARD, nhid=N, tmax=TMAX):
    from contextlib import ExitStack
    import concourse.bass as bass
    import concourse.bacc as bacc
    import concourse.mybir as mybir
    import concourse.tile as tile

    f32 = mybir.dt.float32
    i32 = mybir.dt.int32

    nc = bacc.Bacc("TRN2", debug=False, num_swdge_queues=4)
    table_d = nc.dram_tensor("table", [nhid, F], f32, kind="ExternalInput").ap()
    lat_d = nc.dram_tensor("lat", [vshard, F], f32, kind="ExternalInput").ap()
    nidx_d = nc.dram_tensor("nidx", [vshard, K], i32, kind="ExternalInput").ap()
    bias_d = nc.dram_tensor("bias", [1, F], f32, kind="ExternalInput").ap()
    alpha_d = nc.dram_tensor("alpha", [1, 1], f32, kind="ExternalInput").ap()
    beta_d = nc.dram_tensor("beta", [1, 1], f32, kind="ExternalInput").ap()
    aflow_d = nc.dram_tensor("aflow", [vshard, F], f32, kind="ExternalOutput").ap()
    w_d = nc.dram_tensor("w", [vshard, K], f32, kind="ExternalOutput").ap()

    with tile.TileContext(nc) as tc, ExitStack() as ctx:
        const_pool = ctx.enter_context(tc.tile_pool(name="const", bufs=1))
        io_pool = ctx.enter_context(tc.tile_pool(name="io", bufs=4))
        g_pool = ctx.enter_context(tc.tile_pool(name="gbuf", bufs=4))
        d_pool = ctx.enter_context(tc.tile_pool(name="dbuf", bufs=2))
        sm_pool = ctx.enter_context(tc.tile_pool(name="small", bufs=4))

        bias_sb = const_pool.tile([P, F], f32)
        nc.sync.dma_start(out=bias_sb[:, :], in_=bias_d[:, :].partition_broadcast(P).squeeze(1))
        alpha_sb = const_pool.tile([P, 1], f32)
        nc.sync.dma_start(out=alpha_sb[:, :], in_=alpha_d[:, :].partition_broadcast(P).squeeze(1))
        beta_sb = const_pool.tile([P, 1], f32)
        nc.sync.dma_start(out=beta_sb[:, :], in_=beta_d[:, :].partition_broadcast(P).squeeze(1))
        negbeta_sb = const_pool.tile([P, 1], f32)
        nc.vector.tensor_scalar_mul(negbeta_sb[:, :], beta_sb[:, :], -1.0)
        ab_sb = const_pool.tile([P, 1], f32)
        nc.vector.tensor_mul(ab_sb[:, :], alpha_sb[:, :], beta_sb[:, :])

        for base, T in _tiles_for(vshard, tmax):
            TK = T * K
            TKF = T * K * F
            rows = slice(base, base + P * T)

            idx_t = io_pool.tile([P, TK], i32, tag="idx")
            nc.sync.dma_start(out=idx_t[:, :], in_=nidx_d[rows, :].rearrange("(p t) k -> p (t k)", p=P))
            lat_t = io_pool.tile([P, T * F], f32, tag="lat")
            nc.sync.dma_start(out=lat_t[:, :], in_=lat_d[rows, :].rearrange("(p t) f -> p (t f)", p=P))

            iz_t = sm_pool.tile([P, TK], i32, tag="iz")
            nc.gpsimd.tensor_scalar_max(iz_t[:, :], idx_t[:, :], 0)

            valid_t = sm_pool.tile([P, TK], f32, tag="valid")
            nc.vector.tensor_scalar(valid_t[:, :], idx_t[:, :], -0.5, None, mybir.AluOpType.is_gt)

            g_t = g_pool.tile([P, TKF], f32, tag="G")
            for c in range(TK):
                # one indirect DMA per 128 rows (HW consumes one index per
                # partition); spread across the 4 SWDGE queues - a single
                # qPoolDynamic ring serializes and costs ~8x in wall time
                inst = nc.gpsimd.indirect_dma_start(
                    out=g_t[:, c * F:(c + 1) * F],
                    out_offset=None,
                    in_=table_d[:, :],
                    in_offset=bass.IndirectOffsetOnAxis(ap=iz_t[:, c:c + 1], axis=0),
                )
                if c % 4:
                    inst.ins.queue = f"qPoolDynamic{c % 4}"

            g4 = g_t[:, :].rearrange("p (t k f) -> p t k f", k=K, f=F)
            l4 = lat_t[:, :].rearrange("p (t f) -> p t f", f=F).unsqueeze(2).to_broadcast([P, T, K, F])

            d_t = d_pool.tile([P, TKF], f32, tag="D")
            nc.vector.tensor_tensor(
                out=d_t[:, :].rearrange("p (t k f) -> p t k f", k=K, f=F),
                in0=g4, in1=l4, op=mybir.AluOpType.subtract)
            nc.scalar.square(d_t[:, :], d_t[:, :])
            dist_t = sm_pool.tile([P, TK], f32, tag="dist")
            nc.vector.tensor_reduce(
                out=dist_t[:, :].unsqueeze(2),
                in_=d_t[:, :].rearrange("p (tk f) -> p tk f", f=F),
                axis=mybir.AxisListType.X, op=mybir.AluOpType.add)
            nc.scalar.sqrt(dist_t[:, :], dist_t[:, :])
            nc.vector.tensor_mul(dist_t[:, :], dist_t[:, :], valid_t[:, :])
            rsum_t = sm_pool.tile([P, T], f32, tag="rsum")
            nc.vector.tensor_reduce(
                out=rsum_t[:, :].unsqueeze(2),
                in_=dist_t[:, :].rearrange("p (t k) -> p t k", k=K),
                axis=mybir.AxisListType.X, op=mybir.AluOpType.add)
            recip_t = sm_pool.tile([P, T], f32, tag="recip")
            nc.vector.reciprocal(recip_t[:, :], rsum_t[:, :])
            w_t = sm_pool.tile([P, TK], f32, tag="w")
            nc.vector.tensor_tensor(
                out=w_t[:, :].rearrange("p (t k) -> p t k", k=K),
                in0=dist_t[:, :].rearrange("p (t k) -> p t k", k=K),
                in1=recip_t[:, :].unsqueeze(2).to_broadcast([P, T, K]),
                op=mybir.AluOpType.mult)
            nc.vector.tensor_scalar(w_t[:, :], w_t[:, :], alpha_sb[:, :1], None, mybir.AluOpType.min)
            nc.scalar.activation(w_t[:, :], w_t[:, :], mybir.ActivationFunctionType.Identity,
                                 bias=ab_sb[:, :1], scale=negbeta_sb[:, :1])
            nc.vector.tensor_mul(w_t[:, :], w_t[:, :], valid_t[:, :])
            nc.sync.dma_start(out=w_d[rows, :].rearrange("(p t) k -> p (t k)", p=P), in_=w_t[:, :])

            p_t = d_pool.tile([P, TKF], f32, tag="D")
            nc.vector.tensor_tensor(
                out=p_t[:, :].rearrange("p (t k f) -> p t k f", k=K, f=F),
                in0=g4,
                in1=w_t[:, :].rearrange("p (t k) -> p t k", k=K).unsqueeze(3).to_broadcast([P, T, K, F]),
                op=mybir.AluOpType.mult)
            af_t = io_pool.tile([P, T * F], f32, tag="af")
            nc.vector.tensor_reduce(
                out=af_t[:, :].rearrange("p (t f) -> p t f", f=F),
                in_=p_t[:, :].rearrange("p (t k f) -> p t f k", k=K, f=F),
                axis=mybir.AxisListType.X, op=mybir.AluOpType.add)
            nc.vector.tensor_tensor(
                out=af_t[:, :].rearrange("p (t f) -> p t f", f=F),
                in0=af_t[:, :].rearrange("p (t f) -> p t f", f=F),
                in1=bias_sb[:, :].unsqueeze(1).to_broadcast([P, T, F]),
                op=mybir.AluOpType.add)
            nc.scalar.dma_start(out=aflow_d[rows, :].rearrange("(p t) f -> p (t f)", p=P), in_=af_t[:, :])

    nc.compile()
    return nc


def _get_nc():
    if "nc" not in _cache:
        _cache["nc"] = _build_nc()
    return _cache["nc"]


def _run(in_maps, trace=False):
    from concourse.bass_utils import run_bass_kernel_spmd

    nc = _get_nc()
    return run_bass_kernel_spmd(nc, in_maps, core_ids=list(range(NCORES)), trace=trace)


def make_in_maps(lattice_values, hidden_state, bias, alpha, beta, neighbor_idx):
    lat = np.ascontiguousarray(np.asarray(lattice_values, dtype=np.float32))
    hid = np.ascontiguousarray(np.asarray(hidden_state, dtype=np.float32))
    nidx = np.ascontiguousarray(np.asarray(neighbor_idx, dtype=np.int32))
    bias = np.ascontiguousarray(np.asarray(bias, dtype=np.float32)).reshape(1, F)
    alpha = np.asarray(alpha, dtype=np.float32).reshape(1, 1)
    beta = np.asarray(beta, dtype=np.float32).reshape(1, 1)

    pad = VPAD - N
    lat_p = np.concatenate([lat, np.zeros((pad, F), np.float32)], axis=0)
    nidx_p = np.concatenate([nidx, np.zeros((pad, K), np.int32)], axis=0)

    in_maps = []
    for i in range(NCORES):
        r = slice(i * VSHARD, (i + 1) * VSHARD)
        in_maps.append({
            "table": hid,
            "lat": np.ascontiguousarray(lat_p[r]),
            "nidx": np.ascontiguousarray(nidx_p[r]),
            "bias": bias,
            "alpha": alpha,
            "beta": beta,
        })
    return in_maps


def kernel(lattice_values, hidden_state, bias, alpha, beta, neighbor_idx):
    nidx = np.ascontiguousarray(np.asarray(neighbor_idx, dtype=np.int32))
    in_maps = make_in_maps(lattice_values, hidden_state, bias, alpha, beta, nidx)
    res = _run(in_maps).results
    aflow = np.concatenate([r["aflow"] for r in res], axis=0)[:N]
    w = np.concatenate([r["w"] for r in res], axis=0)[:N]
    return aflow, w, nidx
